# revision 8
# baseline (speedup 1.0000x reference)
"""EvolveGCN (2x GCNConv+GRU + linear head) on 8 Trainium2 NeuronCores.

Strategy: dst-sharded graph parallel, 12500 dst nodes per core (padded to
12544).  All dense per-node compute is feature-major ([128 feat
partitions, nodes free], fp32).  The GCN propagate avoids the slow SWDGE
dma_gather (9ns/row of Pool descriptor-gen) entirely:

  - the transformed node table (rows pre-scaled by dinv[src]) is
    AllGather'd and streamed back window-by-window (one 12544-node rank
    slice at a time, double-buffered) into SBUF;
  - per (window, dst-group) the edge messages are pulled with gpsimd
    ap_gather (free-dim SBUF gather, ~1 col/cycle) in feature-major form;
  - PE transposes 128-edge tiles to edge-major; a banded selection matmul
    (S carries dinv[dst]; dst bands partition each 512-wide group, each
    band <=128 edges per core) scatters them into PSUM; per-window group
    partials are flushed into an SBUF accumulator by the vector engine.

Full fp32 datapath (the correctness gate is tight on small outputs).
"""
import os
import sys
import types

import numpy as np

sys.path.insert(0, "/opt/trn_rl_repo")

N = 100000
E = 1600000
IN = 165
H = 128
NC = 8
SH = 12500
SHP = 12544            # 98 * 128
NWIN = 8               # one window per source rank
NGRP = 25
GRP = 512
GW = [512] * 24 + [256]
SLOTCAP = 128
CHUNK = 8              # gather-call granularity in slots
ZCOL = SHP - 1         # guaranteed-zero table column (node padding)


def _install_ntff_hook():
    if "antenv.axon_hooks" in sys.modules:
        return
    try:
        import antenv
        mod = types.ModuleType("antenv.axon_hooks")
        mod._hook = None
        mod.set_axon_ntff_profile_hook = lambda h: setattr(mod, "_hook", h)
        mod.get_axon_ntff_profile_hook = lambda: mod._hook
        sys.modules["antenv.axon_hooks"] = mod
        antenv.axon_hooks = mod
        from trn_agent_boot.trn_boot import _ntff_profile_via_ctypes
        mod.set_axon_ntff_profile_hook(
            _ntff_profile_via_ctypes("/opt/axon/libaxon_pjrt.so"))
    except Exception:
        pass


def _split_excess_waits(nc, bass, max_waits=1, kinds=("InstDrain",)):
    """This walrus build rejects InstDrain with >1 sem waits; hoist extras
    onto standalone event-semaphore instructions placed just before."""
    wait_op_map = {"sem-ge-imm": "sem-ge", "sem-eq-imm": "sem-eq"}
    for bb in nc.main_func.blocks:
        insts = bb.instructions
        i = 0
        while i < len(insts):
            ins = insts[i]
            if (type(ins).__name__ in kinds and ins.sync_info is not None
                    and len(ins.sync_info.on_wait) > max_waits):
                waits = list(ins.sync_info.on_wait)
                ins.sync_info.on_wait = waits[:max_waits]
                eng = nc.engines[ins.engine]
                new_insts = []
                for w in waits[max_waits:]:
                    sem = bass.SemaphoreHandle(w.ant_name or "s", w.id)
                    bi = eng.wait_op(sem, w.wait_value, wait_op_map[w.wait_mode])
                    popped = None
                    for b2 in nc.main_func.blocks:
                        if b2.instructions and b2.instructions[-1] is bi.ins:
                            popped = b2.instructions.pop()
                            break
                    assert popped is bi.ins
                    new_insts.append(popped)
                for k, ni in enumerate(new_insts):
                    insts.insert(i + k, ni)
                i += len(new_insts)
            i += 1


def _wrap_idx(flat):
    """int16 flat idx stream -> [128, n/16] wrapped layout (16-partition
    wrap, replicated for the 8 gpsimd cores)."""
    assert len(flat) % 16 == 0
    t = np.asarray(flat, np.int16).reshape(-1, 16).T   # [16, n/16]
    return np.tile(t, (8, 1)).copy()                   # [128, n/16]


def _preprocess(edge_index):
    """Build per-core meta streams (gather idx + fp32 S tiles) and the
    uniform per-(window, group) slot structure.

    Returns (dinv, meta[list of [128, TOT] int16], structure) where
    structure[w*NGRP+g] = (mwidth, nidx_cols, nslots, Ws, Bs)."""
    e0 = np.asarray(edge_index[0], dtype=np.int64)
    e1 = np.asarray(edge_index[1], dtype=np.int64)
    deg = np.bincount(e1, minlength=N).astype(np.float64) + 1.0
    dinv = (1.0 / np.sqrt(deg)).astype(np.float32)

    src = np.concatenate([e0, np.arange(N, dtype=np.int64)])
    dst = np.concatenate([e1, np.arange(N, dtype=np.int64)])
    sval = dinv[dst].astype(np.float32)

    src_n = (src // SH) * SHP + (src % SH)
    w_of = src_n // SHP             # window = source rank
    li_of = (src_n % SHP).astype(np.int16)
    core = dst // SH
    dstl = dst % SH

    buckets = []  # buckets[c][(w,g)] = (li, bp, sv)
    for c in range(NC):
        m = core == c
        s_li, s_w = li_of[m], w_of[m]
        s_dl, s_sv = dstl[m], sval[m]
        g = s_dl // GRP
        bp = s_dl - g * GRP
        order = np.lexsort((bp, g, s_w))
        s_li, s_w, g, bp, s_sv = (s_li[order], s_w[order], g[order],
                                  bp[order], s_sv[order])
        key = s_w * NGRP + g
        bnd = np.searchsorted(key, np.arange(NWIN * NGRP + 1))
        d = {}
        for w in range(NWIN):
            for gg in range(NGRP):
                k = w * NGRP + gg
                a, b = bnd[k], bnd[k + 1]
                d[(w, gg)] = (s_li[a:b], bp[a:b], s_sv[a:b])
        buckets.append(d)

    meta_cols = [[] for _ in range(NC)]
    structure = []
    for w in range(NWIN):
        for gg in range(NGRP):
            gw = GW[gg]
            hists = [np.bincount(buckets[c][(w, gg)][1], minlength=gw)
                     for c in range(NC)]
            cums = [np.concatenate([[0], np.cumsum(h)]) for h in hists]
            Bs = [0]
            while Bs[-1] < gw:
                cur = Bs[-1]
                lo, hi = cur + 1, gw
                while lo < hi:
                    mid = (lo + hi + 1) // 2
                    if all(cu[mid] - cu[cur] <= SLOTCAP for cu in cums):
                        lo = mid
                    else:
                        hi = mid - 1
                assert all(cu[lo] - cu[cur] <= SLOTCAP for cu in cums), \
                    f"single-column overflow at (w={w},g={gg})"
                Bs.append(lo)
            nslots = len(Bs) - 1
            Ws = [Bs[j + 1] - Bs[j] for j in range(nslots)]
            nidx_cols = nslots * SLOTCAP // 16
            mwidth = nidx_cols + 2 * sum(Ws)
            structure.append((mwidth, nidx_cols, nslots, Ws, Bs))
            for c in range(NC):
                li, bp, sv = buckets[c][(w, gg)]
                idx_flat = np.full(nslots * SLOTCAP, ZCOL, np.int16)
                s_blk = np.zeros((128, sum(Ws)), np.float32)
                soff = 0
                for j in range(nslots):
                    mm = (bp >= Bs[j]) & (bp < Bs[j + 1])
                    ncnt = int(mm.sum())
                    assert ncnt <= SLOTCAP
                    idx_flat[j * SLOTCAP:j * SLOTCAP + ncnt] = li[mm]
                    s_blk[np.arange(ncnt), soff + (bp[mm] - Bs[j])] = sv[mm]
                    soff += Ws[j]
                idx_w = _wrap_idx(idx_flat)                     # [128, nidx]
                s_i16 = s_blk.view(np.int16).reshape(128, -1)   # [128, 2*sW]
                meta_cols[c].append(np.concatenate([idx_w, s_i16], axis=1))
    meta = [np.ascontiguousarray(np.concatenate(mc, axis=1))
            for mc in meta_cols]
    return dinv, meta, structure


def _build_program(bass, bacc, mybir, tile, structure, tot_meta):
    nc = bacc.Bacc("TRN2", target_bir_lowering=False, debug=False)
    dt = mybir.dt
    f32, i16 = dt.float32, dt.int16

    def din(name, shape, dtype=f32):
        return nc.dram_tensor(name, shape, dtype, kind="ExternalInput").ap()

    xT_hi = din("xT_hi", [128, SHP])
    xT_lo = din("xT_lo", [IN - 128, SHP])
    W0T_hi = din("W0T_hi", [128, H])
    W0T_lo = din("W0T_lo", [IN - 128, H])
    W1T = din("W1T", [H, H])
    WihT = [din(f"WihT{li}", [H, 3 * H]) for li in range(2)]
    WlinT = din("WlinT", [H, 2])
    bcol = [din(f"bcol{li}", [128, 1]) for li in range(2)]
    brc = [din(f"brc{li}", [128, 1]) for li in range(2)]
    bzc = [din(f"bzc{li}", [128, 1]) for li in range(2)]
    bnc = [din(f"bnc{li}", [128, 1]) for li in range(2)]
    bhnc = [din(f"bhnc{li}", [128, 1]) for li in range(2)]
    dinvrow = din("dinvrow", [128, SHP])
    blin_t = din("blin_t", [128, 2])
    meta_d = din("meta_d", [128, tot_meta], i16)

    out = nc.dram_tensor("out", [128, (SHP // 128) * 2], f32,
                         kind="ExternalOutput").ap()
    _dbg = bool(int(os.environ.get("KERNEL_DEBUG", "0")))
    if _dbg:
        dbg_agg = nc.dram_tensor("dbg_agg", [128, SHP], f32,
                                 kind="ExternalOutput").ap()
        dbg_h = nc.dram_tensor("dbg_h", [128, SHP], f32,
                               kind="ExternalOutput").ap()

    tsh = [nc.dram_tensor(f"tshard{li}", [128, SHP], f32) for li in range(2)]
    Ttab = [nc.dram_tensor(f"Ttab{li}", [NC * 128, SHP], f32,
                           addr_space="Shared") for li in range(2)]
    hTd = nc.dram_tensor("hTd", [128, SHP], f32)

    from concourse.masks import make_identity

    maxmw = max(st[0] for st in structure)

    with tile.TileContext(nc) as tc:
        with (
            tc.tile_pool(name="const", bufs=1) as cp,
            tc.tile_pool(name="sb", bufs=2) as sp,
            tc.tile_pool(name="win", bufs=2) as winp,
            tc.tile_pool(name="big", bufs=1) as bigp,
            tc.tile_pool(name="msg", bufs=2) as msgp,
            tc.tile_pool(name="meta", bufs=2) as metp,
            tc.tile_pool(name="mt", bufs=3) as mtp,
            tc.tile_pool(name="ps_tr", bufs=2, space="PSUM") as pp_tr,
            tc.tile_pool(name="ps_agg", bufs=2, space="PSUM") as pp_agg,
            tc.tile_pool(name="ps_g", bufs=2, space="PSUM") as pp_g,
            tc.tile_pool(name="dram", bufs=1, space="DRAM") as _dp,
        ):
            ident = cp.tile([128, 128], f32)
            make_identity(nc, ident[:])
            w0hi = cp.tile([128, H], f32)
            nc.sync.dma_start(out=w0hi[:], in_=W0T_hi[:])
            w0lo = cp.tile([IN - 128, H], f32)
            nc.sync.dma_start(out=w0lo[:], in_=W0T_lo[:])
            w1 = cp.tile([H, H], f32)
            nc.sync.dma_start(out=w1[:], in_=W1T[:])
            wih = [cp.tile([H, 3 * H], f32, name=f"wih{li}")
                   for li in range(2)]
            for li in range(2):
                nc.sync.dma_start(out=wih[li][:], in_=WihT[li][:])
            wlin = cp.tile([H, 2], f32)
            nc.sync.dma_start(out=wlin[:], in_=WlinT[:])
            bc = [cp.tile([128, 1], f32, name=f"bc{li}") for li in range(2)]
            br = [cp.tile([128, 1], f32, name=f"br{li}") for li in range(2)]
            bz = [cp.tile([128, 1], f32, name=f"bz{li}") for li in range(2)]
            bn = [cp.tile([128, 1], f32, name=f"bn{li}") for li in range(2)]
            bhn = [cp.tile([128, 1], f32, name=f"bhn{li}") for li in range(2)]
            for li in range(2):
                nc.sync.dma_start(out=bc[li][:], in_=bcol[li][:])
                nc.sync.dma_start(out=br[li][:], in_=brc[li][:])
                nc.sync.dma_start(out=bz[li][:], in_=bzc[li][:])
                nc.sync.dma_start(out=bn[li][:], in_=bnc[li][:])
                nc.sync.dma_start(out=bhn[li][:], in_=bhnc[li][:])
            blt = cp.tile([128, 2], f32)
            nc.sync.dma_start(out=blt[:], in_=blin_t[:])

            aggT = bigp.tile([128, SHP], f32, tag="aggT")
            y_sb = bigp.tile([128, (SHP // 128) * 2], f32, tag="ysb")

            ACT = mybir.ActivationFunctionType

            def dense(li):
                for g in range(NGRP):
                    gw = GW[g]
                    g0 = g * GRP
                    pt = pp_g.tile([128, GRP], f32, tag="pt")
                    if li == 0:
                        xh = sp.tile([128, GRP], f32, tag="t_a")
                        nc.sync.dma_start(out=xh[:, :gw],
                                          in_=xT_hi[:, g0:g0 + gw])
                        xl = sp.tile([IN - 128, GRP], f32, tag="t_b")
                        nc.sync.dma_start(out=xl[:, :gw],
                                          in_=xT_lo[:, g0:g0 + gw])
                        nc.tensor.matmul(pt[:, :gw], w0hi[:], xh[:, :gw],
                                         start=True, stop=False)
                        nc.tensor.matmul(pt[:, :gw], w0lo[:], xl[:, :gw],
                                         start=False, stop=True)
                    else:
                        hh = sp.tile([128, GRP], f32, tag="t_a")
                        nc.sync.dma_start(out=hh[:, :gw],
                                          in_=hTd[:, g0:g0 + gw])
                        nc.tensor.matmul(pt[:, :gw], w1[:], hh[:, :gw],
                                         start=True, stop=True)
                    dvr = sp.tile([128, GRP], f32, tag="t_c")
                    nc.sync.dma_start(out=dvr[:, :gw],
                                      in_=dinvrow[:, g0:g0 + gw])
                    tsb = sp.tile([128, GRP], f32, tag="t_d")
                    nc.vector.tensor_mul(tsb[:, :gw], pt[:, :gw],
                                         dvr[:, :gw])
                    nc.sync.dma_start(out=tsh[li][:, g0:g0 + gw],
                                      in_=tsb[:, :gw])

            def edge(li):
                si = 0
                moff = 0
                for w in range(NWIN):
                    twin = winp.tile([128, SHP], f32, tag="twin")
                    nc.sync.dma_start(
                        out=twin[:], in_=Ttab[li][w * 128:(w + 1) * 128, :])
                    for g in range(NGRP):
                        gw = GW[g]
                        g0 = g * GRP
                        mwidth, nidx, nslots, Ws, Bs = structure[si]
                        si += 1
                        mt_meta = metp.tile([128, maxmw], i16, tag="meta")
                        nc.sync.dma_start(out=mt_meta[:, :mwidth],
                                          in_=meta_d[:, moff:moff + mwidth])
                        moff += mwidth
                        pagg = pp_agg.tile([128, GRP], f32, tag="pagg")
                        soff = nidx
                        for c0 in range(0, nslots, CHUNK):
                            nch = min(CHUNK, nslots - c0)
                            ncols = nch * SLOTCAP
                            msgs = msgp.tile([128, CHUNK * SLOTCAP], f32,
                                             tag="msgs")
                            nc.gpsimd.ap_gather(
                                out_ap=msgs[:, :ncols],
                                in_ap=twin[:],
                                idxs_ap=mt_meta[
                                    :, c0 * (SLOTCAP // 16):
                                    c0 * (SLOTCAP // 16) + ncols // 16],
                                channels=128, num_elems=SHP, d=1,
                                num_idxs=ncols)
                            for b0 in range(0, nch, 4):
                                nb = min(4, nch - b0)
                                ptr = pp_tr.tile([128, GRP], f32, tag="ptr")
                                for t in range(nb):
                                    nc.tensor.transpose(
                                        out=ptr[:, t * 128:(t + 1) * 128],
                                        in_=msgs[:, (b0 + t) * 128:
                                                 (b0 + t + 1) * 128],
                                        identity=ident[:])
                                mte = mtp.tile([128, GRP], f32, tag="mte")
                                if (b0 // 4) % 2 == 0:
                                    nc.scalar.copy(mte[:, :nb * 128],
                                                   ptr[:, :nb * 128])
                                else:
                                    nc.vector.tensor_copy(mte[:, :nb * 128],
                                                          ptr[:, :nb * 128])
                                for t in range(nb):
                                    j = c0 + b0 + t
                                    Wj = Ws[j]
                                    s_ap = mt_meta[
                                        :, soff:soff + 2 * Wj].bitcast(f32)
                                    soff += 2 * Wj
                                    nc.tensor.matmul(
                                        pagg[:, Bs[j]:Bs[j] + Wj],
                                        mte[:, t * 128:(t + 1) * 128],
                                        s_ap, start=True, stop=True)
                        if w == 0:
                            nc.vector.tensor_copy(aggT[:, g0:g0 + gw],
                                                  pagg[:, :gw])
                        else:
                            nc.vector.tensor_add(aggT[:, g0:g0 + gw],
                                                 aggT[:, g0:g0 + gw],
                                                 pagg[:, :gw])

            def gru(li):
                for g in range(NGRP):
                    gw = GW[g]
                    g0 = g * GRP
                    hg = sp.tile([128, GRP], f32, tag="t_a")
                    nc.scalar.activation(hg[:, :gw], aggT[:, g0:g0 + gw],
                                         ACT.Relu, bias=bc[li][:], scale=1.0)
                    pgr = pp_g.tile([128, GRP], f32, tag="pt")
                    nc.tensor.matmul(pgr[:, :gw], wih[li][:, 0:H],
                                     hg[:, :gw], start=True, stop=True)
                    rt = sp.tile([128, GRP], f32, tag="t_b")
                    nc.scalar.activation(rt[:, :gw], pgr[:, :gw], ACT.Sigmoid,
                                         bias=br[li][:], scale=1.0)
                    pgz = pp_g.tile([128, GRP], f32, tag="pt")
                    nc.tensor.matmul(pgz[:, :gw], wih[li][:, H:2 * H],
                                     hg[:, :gw], start=True, stop=True)
                    zt = sp.tile([128, GRP], f32, tag="t_c")
                    nc.scalar.activation(zt[:, :gw], pgz[:, :gw], ACT.Sigmoid,
                                         bias=bz[li][:], scale=-1.0)
                    pgn = pp_g.tile([128, GRP], f32, tag="pt")
                    nc.tensor.matmul(pgn[:, :gw], wih[li][:, 2 * H:3 * H],
                                     hg[:, :gw], start=True, stop=True)
                    tmp = sp.tile([128, GRP], f32, tag="t_d")
                    nc.vector.tensor_scalar_mul(tmp[:, :gw], rt[:, :gw],
                                                bhn[li][:])
                    st = sp.tile([128, GRP], f32, tag="t_e")
                    nc.vector.tensor_add(st[:, :gw], pgn[:, :gw], tmp[:, :gw])
                    nt = sp.tile([128, GRP], f32, tag="t_f")
                    nc.scalar.activation(nt[:, :gw], st[:, :gw], ACT.Tanh,
                                         bias=bn[li][:], scale=1.0)
                    h2 = sp.tile([128, GRP], f32, tag="t_g")
                    nc.vector.tensor_mul(h2[:, :gw], zt[:, :gw], nt[:, :gw])
                    if li == 0:
                        nc.sync.dma_start(out=hTd[:, g0:g0 + gw],
                                          in_=h2[:, :gw])
                    else:
                        for t in range(gw // 128):
                            py = pp_g.tile([128, 128], f32, tag="py")
                            nc.tensor.matmul(py[:, :2],
                                             h2[:, 128 * t:128 * (t + 1)],
                                             wlin[:], start=True, stop=True)
                            col = g * (GRP // 128) + t
                            nc.vector.tensor_add(
                                y_sb[:, 2 * col:2 * col + 2],
                                py[:, :2], blt[:])

            for li in range(2):
                dense(li)
                nc.gpsimd.collective_compute(
                    "AllGather", mybir.AluOpType.bypass,
                    replica_groups=[list(range(NC))],
                    ins=[tsh[li][:]], outs=[Ttab[li][:]])
                edge(li)
                if _dbg and li == 0:
                    nc.sync.dma_start(out=dbg_agg[:], in_=aggT[:])
                gru(li)
                if _dbg and li == 0:
                    nc.sync.dma_start(out=dbg_h[:], in_=hTd[:])
            nc.sync.dma_start(out=out[:], in_=y_sb[:])

    _split_excess_waits(nc, bass)
    nc.finalize()
    return nc


def kernel(**inputs):
    _install_ntff_hook()
    import concourse.bass as bass
    import concourse.bacc as bacc
    import concourse.mybir as mybir
    import concourse.tile as tile
    from concourse.bass_utils import run_bass_kernel_spmd

    x = np.asarray(inputs["x"], np.float32)
    edge_index = np.asarray(inputs["edge_index"])
    dinv, meta, structure = _preprocess(edge_index)
    tot_meta = sum(st[0] for st in structure)
    assert all(m.shape == (128, tot_meta) for m in meta)

    nc = _build_program(bass, bacc, mybir, tile, structure, tot_meta)

    W0 = np.asarray(inputs["W0"], np.float32)
    W1 = np.asarray(inputs["W1"], np.float32)
    Wlin = np.asarray(inputs["Wlin"], np.float32)
    in_maps = []
    for c in range(NC):
        ids = np.arange(c * SH, (c + 1) * SH)
        xs = np.zeros((SHP, IN), np.float32)
        xs[:SH] = x[ids]
        xT = np.ascontiguousarray(xs.T)
        bias_stage = {}
        for li in range(2):
            bih = np.asarray(inputs[f"bih{li}"], np.float32)
            bhh = np.asarray(inputs[f"bhh{li}"], np.float32)
            bias_stage[f"bcol{li}"] = np.asarray(
                inputs[f"b{li}"], np.float32).reshape(128, 1)
            bias_stage[f"brc{li}"] = (bih[:H] + bhh[:H]).reshape(128, 1)
            bias_stage[f"bzc{li}"] = (
                -(bih[H:2 * H] + bhh[H:2 * H])).reshape(128, 1)
            bias_stage[f"bnc{li}"] = bih[2 * H:].reshape(128, 1)
            bias_stage[f"bhnc{li}"] = bhh[2 * H:].reshape(128, 1)
        dv = np.zeros(SHP, np.float32)
        dv[:SH] = dinv[ids]
        in_maps.append({
            "xT_hi": np.ascontiguousarray(xT[:128]),
            "xT_lo": np.ascontiguousarray(xT[128:]),
            "W0T_hi": np.ascontiguousarray(W0.T[:128]),
            "W0T_lo": np.ascontiguousarray(W0.T[128:]),
            "W1T": np.ascontiguousarray(W1.T),
            "WihT0": np.ascontiguousarray(
                np.asarray(inputs["Wih0"], np.float32).T),
            "WihT1": np.ascontiguousarray(
                np.asarray(inputs["Wih1"], np.float32).T),
            "WlinT": np.ascontiguousarray(Wlin.T),
            **bias_stage,
            "dinvrow": np.tile(dv, (128, 1)),
            "blin_t": np.tile(np.asarray(inputs["blin"], np.float32),
                              (128, 1)),
            "meta_d": meta[c],
        })

    res = run_bass_kernel_spmd(nc, in_maps, list(range(NC)),
                               trace=bool(int(os.environ.get(
                                   "KERNEL_TRACE", "0"))))
    kernel.last_results = res
    y = np.zeros((N, 2), np.float32)
    for c in range(NC):
        o = res.results[c]["out"]  # [128, 98*2]
        yy = o.reshape(128, SHP // 128, 2).transpose(1, 0, 2).reshape(SHP, 2)
        y[c * SH:(c + 1) * SH] = yy[:SH]
    return y
